# revision 3
# baseline (speedup 1.0000x reference)
"""Trainium2 Bass kernel for nn_ContextAttentionAdapterWrapper — v4.

Same math as v3 (track-sharded kv/attention, replicated q, v-half folded to
8 per-head vw columns, LN folds, f16 matmul paths) with a pipelined
schedule:

  ctx/kv first (PE), then q stats, then q-projection emitted it-by-it with
  that head-pair's attention (sim -> exp -> reduces) interleaved behind it,
  so Scalar/Vector/GpSimd run the softmax tail while the PE is still
  projecting later head pairs.

Vector-work cuts vs v3:
  - j-axis reduction is two-stage: GpSimd adds the two j-halves (f16),
    Vector reduces the 35-wide halves into f32.
  - numer product goes to a separate tile (no WAR stall on the denominator).
  - all 448 divisions batched at the end via reciprocal_approx_fast.
"""

import os
import sys

for _p in ("/opt/trn_rl_repo", "/root/.axon_site/_ro/trn_rl_repo"):
    if os.path.isdir(_p) and _p not in sys.path:
        sys.path.append(_p)

import numpy as np

import concourse.bass as bass
import concourse.tile as tile
from concourse import bacc, mybir
from concourse.bass_utils import run_bass_kernel_spmd
from concourse.tile_rust import add_dep_helper

F32 = mybir.dt.float32
F16 = mybir.dt.float16
AF = mybir.ActivationFunctionType
AX = mybir.AxisListType
OP = mybir.AluOpType

N_CORES = 8
B, N, DH = 1, 896, 3072
C, J, DC = 64, 127, 1024
H, D = 8, 64
INNER = H * D
EPS = 1e-5
SCALE = D ** -0.5
TPC = C // N_CORES
KQ = DH // 128
KKV = DC // 128
NIQ = INNER // 128
NKH = NIQ
NT = N // 128

_BUILD_CACHE = {}
LAST_RESULTS = None


def _dep(after, *befores):
    for b in befores:
        add_dep_helper(after.ins, b.ins, sync=True, reason="dram bounce order")


def _build(jeff):
    jp = jeff + 1
    TJ = TPC * jp
    TJ0 = TPC * jeff
    jh = jp // 2
    even = (jp % 2 == 0)
    nc = bacc.Bacc("TRN2", target_bir_lowering=False, debug=False,
                   enable_asserts=False, num_devices=N_CORES)

    embT = nc.dram_tensor("embT", [KQ, 128, N], F16, kind="ExternalInput").ap()
    wq = nc.dram_tensor("wq", [NIQ, 128, KQ, 128], F16, kind="ExternalInput").ap()
    augq = nc.dram_tensor("augq", [2, NIQ, 128], F16, kind="ExternalInput").ap()
    onesw = nc.dram_tensor("onesw", [128, KQ, 2], F16, kind="ExternalInput").ap()
    wkv = nc.dram_tensor("wkv", [128, NKH, KKV, 128], F16, kind="ExternalInput").ap()
    augkv = nc.dram_tensor("augkv", [2, NKH, 128], F16, kind="ExternalInput").ap()
    wv = nc.dram_tensor("wv", [128, KKV, H], F16, kind="ExternalInput").ap()
    augv = nc.dram_tensor("augv", [2, H], F16, kind="ExternalInput").ap()
    ctxT = nc.dram_tensor("ctxT", [128, KKV, TJ0], F16, kind="ExternalInput").ap()
    nullk = nc.dram_tensor("nullk", [128, NKH], F16, kind="ExternalInput").ap()
    cnv = nc.dram_tensor("cnv", [1, H], F32, kind="ExternalInput").ap()
    consts = nc.dram_tensor("consts", [1, 4], F32, kind="ExternalInput").ap()
    out_d = nc.dram_tensor("out", [N, TPC], F32, kind="ExternalOutput").ap()

    from contextlib import ExitStack
    with tile.TileContext(nc) as tc, ExitStack() as ctx:
        const = ctx.enter_context(tc.tile_pool(name="const", bufs=1))
        dram = ctx.enter_context(tc.tile_pool(name="dram", bufs=1, space="DRAM"))
        sq_pool = ctx.enter_context(tc.tile_pool(name="sqp", bufs=2))
        att = ctx.enter_context(tc.tile_pool(name="att", bufs=3))
        ps_fe = ctx.enter_context(tc.tile_pool(name="ps_fe", bufs=3, space="PSUM"))
        ps_sim = ctx.enter_context(tc.tile_pool(name="ps_sim", bufs=2, space="PSUM"))

        # ---------------- loads (kv side first) ----------------
        ctx_sb = const.tile([128, KKV, 2, TJ0], F16)
        nc.sync.dma_start(out=ctx_sb[:, :, 0, :], in_=ctxT[:])
        wkv_sb = const.tile([128, NKH, KKV, 128], F16)
        nc.sync.dma_start(out=wkv_sb[:], in_=wkv[:])
        wv_sb = const.tile([128, KKV, H], F16)
        nc.sync.dma_start(out=wv_sb[:], in_=wv[:])
        augkv_sb = const.tile([1, 2, NKH, 128], F16)
        nc.sync.dma_start(out=augkv_sb[:], in_=augkv.rearrange("r t i -> (r t i)"))
        augv_sb = const.tile([1, 2, H], F16)
        nc.sync.dma_start(out=augv_sb[:], in_=augv.rearrange("r h -> (r h)"))
        cnv_sb = const.tile([128, H], F32)
        nc.sync.dma_start(out=cnv_sb[:],
                          in_=bass.AP(tensor=cnv.tensor, offset=cnv.offset,
                                      ap=[[0, 128], [1, H]]))
        consts_sb = const.tile([128, 4], F32)
        nc.sync.dma_start(out=consts_sb[:],
                          in_=bass.AP(tensor=consts.tensor, offset=consts.offset,
                                      ap=[[0, 128], [1, 4]]))
        onec = const.tile([128, 1], F16)
        nc.vector.memset(onec[:], 1.0)
        emb_sb = const.tile([128, KQ, N], F16)
        for k in range(KQ):
            nc.sync.dma_start(out=emb_sb[:, k, :], in_=embT[k])
        onesw_sb = const.tile([128, KQ, 2], F16)
        nc.sync.dma_start(out=onesw_sb[:], in_=onesw[:])
        wq_sb = const.tile([128, NIQ, KQ, 128], F16)
        for it in range(NIQ):
            nc.sync.dma_start(out=wq_sb[:, it], in_=wq[it])
        augq_sb = const.tile([1, 2, NIQ, 128], F16)
        nc.sync.dma_start(out=augq_sb[:], in_=augq.rearrange("r t i -> (r t i)"))

        # ---------------- ctx stats ----------------
        nc.vector.tensor_mul(ctx_sb[:, :, 1, :], ctx_sb[:, :, 0, :],
                             ctx_sb[:, :, 0, :])
        crows = const.tile([1, 3, TJ0], F32)
        cmu_r = crows[0:1, 0, :]
        cvar_r = crows[0:1, 1, :]
        csig_r = crows[0:1, 2, :]
        csc_r = cvar_r
        cch = [(o, min(256, TJ0 - o)) for o in range(0, TJ0, 256)]
        for (o, w) in cch:
            cst = ps_fe.tile([128, 512], F32, name="fe_ps")[0:1, :]
            for k in range(KKV):
                nc.tensor.matmul(cst[:, 0:2 * w].rearrange("p (r x) -> p r x", r=2),
                                 onec[:], ctx_sb[:, k, :, o:o + w],
                                 start=(k == 0), stop=(k == KKV - 1))
            nc.scalar.mul(cmu_r[:, o:o + w], cst[:, 0:w], 1.0 / DC)
            nc.scalar.mul(cvar_r[:, o:o + w], cst[:, w:2 * w], 1.0 / DC)
        nc.vector.tensor_mul(csig_r, cmu_r, cmu_r)
        nc.vector.tensor_sub(cvar_r, cvar_r, csig_r)
        nc.scalar.activation(out=csig_r, in_=cvar_r, func=AF.Sqrt,
                             bias=consts_sb[0:1, 0:1])
        nc.vector.reciprocal_approx_fast(csc_r, csig_r)
        cmu16 = const.tile([1, TJ0], F16)
        nc.vector.tensor_copy(cmu16[:], cmu_r)
        csig16 = const.tile([1, TJ0], F16)
        nc.vector.tensor_copy(csig16[:], csig_r)
        sc_bc = const.tile([128, TJ0], F32)
        nc.gpsimd.partition_broadcast(sc_bc[:], csc_r, channels=128)

        # ---------------- kv projection + vw rows ----------------
        kvT = const.tile([128, NKH, TPC, jp], F16)
        nc.vector.memset(kvT[:], 0.0)
        tch = [(0, 4), (4, 4)]
        for it in range(NKH):
            for (to, tw) in tch:
                fo, fw = to * jeff, tw * jeff
                kv_ps = ps_fe.tile([128, 512], F32, name="fe_ps")[:, :fw]
                for k in range(KKV):
                    nc.tensor.matmul(kv_ps, wkv_sb[:, it, k, :],
                                     ctx_sb[:, k, 0, fo:fo + fw],
                                     start=(k == 0), stop=False)
                nc.tensor.matmul(kv_ps, augkv_sb[0:1, 0, it, :],
                                 cmu16[:, fo:fo + fw], start=False, stop=False)
                nc.tensor.matmul(kv_ps, augkv_sb[0:1, 1, it, :],
                                 csig16[:, fo:fo + fw], start=False, stop=True)
                nc.vector.tensor_mul(
                    kvT[:, it, to:to + tw, 1:jp],
                    kv_ps.rearrange("p (t j) -> p t j", t=tw),
                    sc_bc[:, fo:fo + fw].rearrange("p (t j) -> p t j", t=tw))
        nsrc = bass.AP(tensor=nullk.tensor, offset=nullk.offset,
                       ap=[[NKH, 128], [1, NKH], [0, TPC]])
        nc.sync.dma_start(out=kvT[:, :, :, 0], in_=nsrc)

        vw_row = const.tile([8, TPC, jp], F16)
        for (to, tw) in tch:
            fo, fw = to * jeff, tw * jeff
            vw_ps = ps_fe.tile([128, 512], F32, name="fe_ps")[0:H, :fw]
            for k in range(KKV):
                nc.tensor.matmul(vw_ps, wv_sb[:, k, :],
                                 ctx_sb[:, k, 0, fo:fo + fw],
                                 start=(k == 0), stop=False)
            nc.tensor.matmul(vw_ps, augv_sb[0:1, 0, :], cmu16[:, fo:fo + fw],
                             start=False, stop=False)
            nc.tensor.matmul(vw_ps, augv_sb[0:1, 1, :], csig16[:, fo:fo + fw],
                             start=False, stop=True)
            nc.vector.tensor_mul(
                vw_row[:, to:to + tw, 1:jp],
                vw_ps.rearrange("p (t j) -> p t j", t=tw),
                sc_bc[0:H, fo:fo + fw].rearrange("p (t j) -> p t j", t=tw))
        vw_d = dram.tile([H, TPC * jp], F16)
        w_vw = nc.sync.dma_start(out=vw_d[:],
                                 in_=vw_row[:].rearrange("p t j -> p (t j)"))
        vw0 = const.tile([1, H, TPC, jp], F16)
        w_vw0 = nc.sync.dma_start(out=vw0[:].rearrange("p h t j -> p (h t j)"),
                                  in_=vw_d.rearrange("h x -> (h x)"))
        _dep(w_vw0, w_vw)
        zr = const.tile([1, 16], F32)
        nc.vector.memset(zr[:], 0.0)
        for h in range(H):
            nc.vector.tensor_scalar_add(vw0[0:1, h, :, 0], zr[0:1, 0:TPC],
                                        cnv_sb[0:1, h:h + 1])
        vw_b = const.tile([128, H, TPC, jp], F16)
        nc.gpsimd.partition_broadcast(
            vw_b[:].rearrange("p h t j -> p (h t j)"),
            vw0[:].rearrange("p h t j -> p (h t j)"), channels=128)

        # ---------------- q stats ----------------
        qch = [(o, min(256, N - o)) for o in range(0, N, 256)]
        rows = const.tile([1, 4, N], F32)
        mu_r = rows[0:1, 0, :]
        var_r = rows[0:1, 1, :]
        sig_r = rows[0:1, 2, :]
        s_r = rows[0:1, 3, :]
        ep2 = const.tile([2, N], F32)
        KH2 = KQ // 2
        for (o, w) in qch:
            sl = slice(o, o + w)
            sum_ps = ps_fe.tile([128, 512], F32, name="fe_ps")[0:2, 0:w]
            sq_ps = ps_fe.tile([128, 512], F32, name="fe_ps")[0:1, 0:w]
            for g in range(2):
                sq = sq_pool.tile([128, KH2, 256], F16, name="sq")[:, :, :w]
                nc.vector.tensor_mul(sq[:], emb_sb[:, g * KH2:(g + 1) * KH2, sl],
                                     emb_sb[:, g * KH2:(g + 1) * KH2, sl])
                for kk in range(KH2):
                    k = g * KH2 + kk
                    nc.tensor.matmul(sum_ps, onesw_sb[:, k, :],
                                     emb_sb[:, k, sl],
                                     start=(k == 0), stop=(k == KQ - 1))
                    nc.tensor.matmul(sq_ps, onec[:], sq[:, kk, :],
                                     start=(k == 0), stop=(k == KQ - 1))
            nc.scalar.mul(mu_r[:, sl], sum_ps[0:1, :], 1.0 / DH)
            nc.scalar.mul(var_r[:, sl], sq_ps, 1.0 / DH)
            nc.vector.tensor_copy(ep2[0:2, sl], sum_ps[0:2, :])
        nc.vector.tensor_scalar_add(ep2[0:2, :], ep2[0:2, :],
                                    consts_sb[0:2, 1:2])
        nc.vector.tensor_mul(sig_r, mu_r, mu_r)
        nc.vector.tensor_sub(var_r, var_r, sig_r)
        nc.scalar.activation(out=sig_r, in_=var_r, func=AF.Sqrt,
                             bias=consts_sb[0:1, 0:1])
        nc.vector.reciprocal_approx_fast(s_r, sig_r)
        nc.scalar.mul(s_r, s_r, SCALE)
        mu16 = const.tile([1, N], F16)
        nc.vector.tensor_copy(mu16[:], mu_r)
        sig16 = const.tile([1, N], F16)
        nc.vector.tensor_copy(sig16[:], sig_r)
        s_bc = const.tile([128, N], F32)
        nc.gpsimd.partition_broadcast(s_bc[:], s_r, channels=128)
        ep_d = dram.tile([N], F32)
        w_ep = nc.sync.dma_start(out=ep_d[:], in_=ep2[1:2, :])
        ep_col = const.tile([128, NT], F32)
        w_epc = nc.sync.dma_start(
            out=ep_col[:],
            in_=bass.AP(tensor=ep_d.tensor, offset=ep_d.offset,
                        ap=[[1, 128], [128, NT]]))
        _dep(w_epc, w_ep)

        # ---------------- q projection x attention (pipelined by it) -------
        qT_sb = const.tile([128, NIQ, N], F16)
        nd_all = const.tile([128, 2, NT, H, TPC], F32)
        kvf = kvT.rearrange("p i t j -> p i (t j)")
        sch = [(0, min(512, TJ))] + ([(512, TJ - 512)] if TJ > 512 else [])
        pch = [(0, 448), (448, 448)]

        def emit_qproj(it, o, w):
            sl = slice(o, o + w)
            q_ps = ps_fe.tile([128, 512], F32, name="fe_ps")[:, :w]
            for k in range(KQ):
                nc.tensor.matmul(q_ps, wq_sb[:, it, k, :], emb_sb[:, k, sl],
                                 start=(k == 0), stop=False)
            nc.tensor.matmul(q_ps, augq_sb[0:1, 0, it, :], mu16[:, sl],
                             start=False, stop=False)
            nc.tensor.matmul(q_ps, augq_sb[0:1, 1, it, :], sig16[:, sl],
                             start=False, stop=True)
            nc.vector.tensor_mul(qT_sb[:, it, sl], q_ps, s_bc[:, sl])

        def emit_att(it, i):
            """Attention for head pair (2it, 2it+1) on i-tile i."""
            isl = slice(i * 128, (i + 1) * 128)
            exp_sb = att.tile([128, 2, TPC, jp], F16, name="exp_sb")
            for sub in range(2):
                h = 2 * it + sub
                po = 64 * (h % 2)
                sim_ps = ps_sim.tile([128, 1024], F32, name="sim_ps")
                for (o, w) in sch:
                    nc.tensor.matmul(sim_ps[:, o:o + w],
                                     qT_sb[po:po + 64, it, isl],
                                     kvf[po:po + 64, it, o:o + w],
                                     start=True, stop=True)
                nc.scalar.activation(
                    out=exp_sb[:, sub].rearrange("p t j -> p (t j)"),
                    in_=sim_ps[:, 0:TJ], func=AF.Exp)
            prod = att.tile([128, 2, TPC, jp], F16, name="prod")
            nc.vector.tensor_mul(prod[:], exp_sb[:], vw_b[:, 2 * it:2 * it + 2])
            if even:
                dh_t = att.tile([128, 2, TPC, jh], F16, name="dh")
                nc.gpsimd.tensor_add(dh_t[:], exp_sb[:, :, :, 0:jh],
                                     exp_sb[:, :, :, jh:jp])
                nh_t = att.tile([128, 2, TPC, jh], F16, name="nh")
                nc.gpsimd.tensor_add(nh_t[:], prod[:, :, :, 0:jh],
                                     prod[:, :, :, jh:jp])
                nc.vector.tensor_reduce(nd_all[:, 1, i, 2 * it:2 * it + 2],
                                        dh_t[:], axis=AX.X, op=OP.add)
                nc.vector.tensor_reduce(nd_all[:, 0, i, 2 * it:2 * it + 2],
                                        nh_t[:], axis=AX.X, op=OP.add)
            else:
                nc.vector.tensor_reduce(nd_all[:, 1, i, 2 * it:2 * it + 2],
                                        exp_sb[:], axis=AX.X, op=OP.add)
                nc.vector.tensor_reduce(nd_all[:, 0, i, 2 * it:2 * it + 2],
                                        prod[:], axis=AX.X, op=OP.add)

        # schedule: qproj chunks feed sims three i-tiles at a time
        emit_qproj(0, *pch[0])
        for it in range(NIQ):
            if it + 1 < NIQ:
                nxt = it + 1
            for i in range(3):
                emit_att(it, i)
            emit_qproj(it, *pch[1])
            if it + 1 < NIQ:
                emit_qproj(it + 1, *pch[0])
            for i in range(3, NT):
                emit_att(it, i)

        # ---------------- batched epilogue ----------------
        nden = nd_all[:, 1].rearrange("p i h t -> p (i h t)")
        nnum = nd_all[:, 0].rearrange("p i h t -> p (i h t)")
        nc.vector.reciprocal_approx_fast(nden, nden)
        nc.vector.tensor_mul(nnum, nnum, nden)
        res = const.tile([128, NT, TPC], F32)
        nc.vector.tensor_reduce(res[:],
                                nd_all[:, 0].rearrange("p i h t -> p i t h"),
                                axis=AX.X, op=OP.add)
        for i in range(NT):
            nc.vector.tensor_scalar_add(res[:, i], res[:, i], ep_col[:, i:i + 1])
        sp = const.tile([128, NT, TPC], F32)
        nc.scalar.activation(out=sp[:], in_=res[:], func=AF.Exp)
        nc.vector.tensor_scalar_add(sp[:], sp[:], 1.0)
        nc.scalar.activation(out=sp[:], in_=sp[:], func=AF.Ln)
        for i in range(NT):
            nc.sync.dma_start(out=out_d[i * 128:(i + 1) * 128, :], in_=sp[:, i])

    nc.compile()
    return nc


def _prep(inputs):
    f16 = np.float16
    emb = np.asarray(inputs["embeddings"], np.float32)[0]
    ctxf = np.asarray(inputs["context"], np.float32)
    km = np.asarray(inputs["context_mask"])[0].astype(bool)
    Wq = np.asarray(inputs["Wq"], np.float32)
    Wkv = np.asarray(inputs["Wkv"], np.float32)
    Wo = np.asarray(inputs["Wo"], np.float32)
    Wp = np.asarray(inputs["Wp"], np.float32)
    qg = np.asarray(inputs["q_gamma"], np.float32)
    qb = np.asarray(inputs["q_beta"], np.float32)
    kg = np.asarray(inputs["kv_gamma"], np.float32)
    kb = np.asarray(inputs["kv_beta"], np.float32)
    nk = np.asarray(inputs["null_k"], np.float32)
    nv = np.asarray(inputs["null_v"], np.float32)
    bo = np.asarray(inputs["bo"], np.float32)
    bp = np.asarray(inputs["bp"], np.float32)

    jeff = int(km.sum())
    assert jeff >= 1

    wop = (Wo @ Wp)[:, 0]
    c0 = np.float32(bo @ Wp[:, 0] + bp[0])
    Wq_f = qg[:, None] * Wq
    qbias = qb @ Wq
    qcol = Wq_f.sum(0)
    Wkv_f = kg[:, None] * Wkv
    kvbias = kb @ Wkv
    Wk_f = Wkv_f[:, :INNER]
    kcol = Wk_f.sum(0)
    kbias = kvbias[:INNER]
    Wv_f = Wkv_f[:, INNER:]
    wv = np.stack([Wv_f[:, h * D:(h + 1) * D] @ wop[h * D:(h + 1) * D]
                   for h in range(H)], axis=1)
    vbias = np.array([kvbias[INNER + h * D:INNER + (h + 1) * D]
                      @ wop[h * D:(h + 1) * D] for h in range(H)], np.float32)
    vcol = wv.sum(0)
    cnv = np.array([nv[h * D:(h + 1) * D] @ wop[h * D:(h + 1) * D]
                    for h in range(H)], np.float32)

    wq_t = np.ascontiguousarray(
        Wq_f.reshape(KQ, 128, NIQ, 128).transpose(2, 1, 0, 3)).astype(f16)
    augq = np.ascontiguousarray(np.stack(
        [-qcol, qbias]).reshape(2, NIQ, 128)).astype(f16)
    onesw = np.empty((128, KQ, 2), np.float32)
    onesw[:, :, 0] = 1.0
    onesw[:, :, 1] = Wp[:, 0].reshape(KQ, 128).T
    wkv_t = np.ascontiguousarray(
        Wk_f.reshape(KKV, 128, NKH, 128).transpose(1, 2, 0, 3)).astype(f16)
    augkv = np.ascontiguousarray(np.stack(
        [-kcol, kbias]).reshape(2, NKH, 128)).astype(f16)
    wv_t = np.ascontiguousarray(
        wv.reshape(KKV, 128, H).transpose(1, 0, 2)).astype(f16)
    augv = np.stack([-vcol, vbias]).astype(f16)
    nullk_t = np.ascontiguousarray(nk.reshape(NKH, 128).T).astype(f16)
    consts = np.array([[EPS, c0, 0.0, 0.0]], np.float32)

    embT = np.ascontiguousarray(
        emb.reshape(N, KQ, 128).transpose(1, 2, 0)).astype(f16)
    ctx_kept = ctxf[:, km, :]
    shared = {
        "embT": embT, "wq": wq_t, "augq": augq, "onesw": onesw.astype(f16),
        "wkv": wkv_t, "augkv": augkv, "wv": wv_t, "augv": augv,
        "nullk": nullk_t, "cnv": cnv[None], "consts": consts,
    }
    in_maps = []
    for m in range(N_CORES):
        sh = ctx_kept[m * TPC:(m + 1) * TPC]
        ctxT = np.ascontiguousarray(
            sh.reshape(TPC, jeff, KKV, 128).transpose(3, 2, 0, 1)).astype(f16)
        in_maps.append(dict(shared, ctxT=ctxT.reshape(128, KKV, TPC * jeff)))
    return jeff, in_maps


def kernel(**inputs) -> np.ndarray:
    global LAST_RESULTS
    jeff, in_maps = _prep(inputs)
    if jeff not in _BUILD_CACHE:
        _BUILD_CACHE[jeff] = _build(jeff)
    nc = _BUILD_CACHE[jeff]
    trace = os.environ.get("BASS_KERNEL_TRACE", "") == "1"
    res = run_bass_kernel_spmd(nc, in_maps, core_ids=list(range(N_CORES)),
                               trace=trace)
    LAST_RESULTS = res
    out = np.concatenate([res.results[m]["out"] for m in range(N_CORES)],
                         axis=1)
    return out[None].astype(np.float32)
